# revision 11
# baseline (speedup 1.0000x reference)
"""Trainium2 Bass kernel for HIPA block (pool-pyramid importance mask + proj).

Contract: kernel(**inputs) takes FULL inputs (B=8 batch), returns the FULL
outputs (out_sparse, kept_feats, kept_coords, keep). Internally shards
data-parallel over batch across 8 NeuronCores (1 sample per core).

Per-core device program:
  pass 1: grid max-pool x (256,224,224) -> coarse (256,4,4)   [reads x once]
  small:  LayerNorm stats + importance + threshold mask + proj (all tiny)
  mask:   bilinear 4x4 -> 224x224 built via two K<=4 matmuls (rank-4 structure)
  pass 2: out = x * mask (row-partition layout, broadcast over channels)
          [reads x again, writes out]
Host: folds gamma/beta/w_proj/b_proj into wg/A/Bw, computes kept_coords from
the device keep mask.
"""

import numpy as np

B, C, H, W = 8, 256, 224, 224
OUT = 256
G = 4              # coarse grid (2 ** (NUM_LEVELS-1))
BLK = H // G       # 56 rows per pooling block
RSUB = 14          # rows per pass-1 tile (4 tiles per pooling block)
NSUB = BLK // RSUB  # 4 sub-tiles per pooling block
RC = 112           # rows per pass-2 chunk (2 chunks cover H)
CSUB = 16          # channels per pass-2 tile
LN_EPS = 1e-5
N_CORES = 8

_CACHE = {}


def _bilinear_wt():
    """(G, W) f32: WT[i, r] = bilinear (half-pixel, clamped) weight of coarse
    row i for output row r. Matches jax.image.resize(method='bilinear') for
    4 -> 224 upsampling (antialias is a no-op when upsampling)."""
    wt = np.zeros((G, W), np.float64)
    scale = G / W
    for r in range(W):
        sample = (r + 0.5) * scale - 0.5
        for i in range(G):
            wt[i, r] = max(0.0, 1.0 - abs(sample - i))
        wt[:, r] /= wt[:, r].sum()
    return wt.astype(np.float32)


def _build_nc(rsub=RSUB, csub=CSUB, p1bufs=8, p2bufs=4):
    import concourse.bass as bass
    import concourse.bacc as bacc
    import concourse.tile as tile
    from concourse import mybir
    from contextlib import ExitStack

    f32 = mybir.dt.float32
    X = mybir.AxisListType.X
    mx_op = mybir.AluOpType.max

    # Bacc (not raw Bass): its compile() pass legalizes multi-wait
    # instructions (walrus allows only one sync wait per DMA).
    nc = bacc.Bacc("TRN2", target_bir_lowering=False, debug=False)

    x_d = nc.dram_tensor("x", [C, H, W], f32, kind="ExternalInput")
    wg_d = nc.dram_tensor("wg", [C, OUT], f32, kind="ExternalInput")
    ab_d = nc.dram_tensor("ab", [2, OUT], f32, kind="ExternalInput")
    sc_d = nc.dram_tensor("sc", [1, 1], f32, kind="ExternalInput")
    out_d = nc.dram_tensor("out", [C, H, W], f32, kind="ExternalOutput")
    kept_d = nc.dram_tensor("kept", [G * G, OUT], f32, kind="ExternalOutput")
    keep_d = nc.dram_tensor("keep", [1, G * G], f32, kind="ExternalOutput")

    wt = _bilinear_wt()
    wxT_d = nc.inline_tensor(np.ascontiguousarray(wt.reshape(1, G, W)), name="wxTc")
    wyT_d = nc.inline_tensor(np.ascontiguousarray(wt), name="wyTc")
    i128_d = nc.inline_tensor(np.eye(128, dtype=np.float32), name="i128c")

    x_ap = x_d.ap()
    out_ap = out_d.ap()

    with tile.TileContext(nc) as tc, ExitStack() as ctx:
        consts = ctx.enter_context(tc.tile_pool(name="consts", bufs=1))
        small = ctx.enter_context(tc.tile_pool(name="small", bufs=1))
        psum = ctx.enter_context(tc.tile_pool(name="psum", bufs=1, space="PSUM"))
        # x1 bufs=8 matches the 8 DMA sem lanes: the slot-WAW predecessor of
        # each load is its own-lane predecessor, so Tile emits at most 2
        # sync waits per DMA (the DIRECT2D encoding can't hold more).
        p1 = ctx.enter_context(tc.tile_pool(name="p1", bufs=p1bufs))
        p1s = ctx.enter_context(tc.tile_pool(name="p1s", bufs=p1bufs))
        p2 = ctx.enter_context(tc.tile_pool(name="p2", bufs=p2bufs))

        # ---- constants / params to SBUF
        i_sb = consts.tile([128, 128], f32)
        nc.sync.dma_start(out=i_sb, in_=i128_d.ap())
        wxT_sb = consts.tile([1, G, W], f32)
        nc.sync.dma_start(out=wxT_sb, in_=wxT_d.ap())
        wyT_sb = consts.tile([G, W], f32)
        nc.sync.dma_start(out=wyT_sb, in_=wyT_d.ap())
        wg_sb = consts.tile([128, 2, OUT], f32)
        for cc in range(2):
            nc.sync.dma_start(out=wg_sb[:, cc, :],
                              in_=wg_d.ap()[cc * 128:(cc + 1) * 128, :])
        a_sb = consts.tile([G * G, OUT], f32)
        r0 = ab_d.ap()[0:1, :]
        nc.sync.dma_start(out=a_sb, in_=bass.AP(tensor=r0.tensor, offset=r0.offset,
                                                ap=[[0, G * G], [1, OUT]]))
        b_sb = consts.tile([G * G, OUT], f32)
        r1 = ab_d.ap()[1:2, :]
        nc.sync.dma_start(out=b_sb, in_=bass.AP(tensor=r1.tensor, offset=r1.offset,
                                                ap=[[0, G * G], [1, OUT]]))
        thr_sb = small.tile([1, 1], f32)
        nc.sync.dma_start(out=thr_sb, in_=sc_d.ap())

        # ---- pass 1: grid max-pool -> coarse (2 chunks of (128, 4, 4))
        coarse = []
        for cc in range(2):
            ct = small.tile([128, G, G], f32, name=f"coarse{cc}")
            coarse.append(ct)
        nsub = BLK // rsub
        for cc in range(2):
            for band in range(G):
                hms = []
                for sub in range(nsub):
                    row0 = band * BLK + sub * rsub
                    x1 = p1.tile([128, rsub, W], f32, tag="x1", name="x1")
                    nc.sync.dma_start(
                        out=x1,
                        in_=x_ap[cc * 128:(cc + 1) * 128, row0:row0 + rsub, :])
                    p1t = p1s.tile([128, rsub * G], f32, tag="p1t", name="p1t")
                    nc.vector.tensor_reduce(
                        out=p1t, in_=x1.rearrange("p r (j w) -> p (r j) w", j=G),
                        axis=X, op=mx_op)
                    hm = p1s.tile([128, G], f32, tag="hm", name="hm")
                    nc.vector.tensor_reduce(
                        out=hm, in_=p1t.rearrange("p (r j) -> p j r", j=G),
                        axis=X, op=mx_op)
                    hms.append(hm)
                while len(hms) > 1:
                    nxt = []
                    for a in range(0, len(hms) - 1, 2):
                        if len(hms) == 2:
                            nc.vector.tensor_max(out=coarse[cc][:, band, :],
                                                 in0=hms[a], in1=hms[a + 1])
                            nxt.append(None)
                        else:
                            hmx = p1s.tile([128, G], f32, tag=f"hmx{a}",
                                           name=f"hmx{a}")
                            nc.vector.tensor_max(out=hmx, in0=hms[a],
                                                 in1=hms[a + 1])
                            nxt.append(hmx)
                    if len(hms) % 2:
                        nxt.append(hms[-1])
                    if nxt == [None]:
                        break
                    hms = nxt

        # ---- feat = coarse^T (16, 256) via PE transpose; LN stats
        featp = []
        for cc in range(2):
            fp = psum.tile([G * G, 128], f32, name=f"featp{cc}")
            nc.tensor.matmul(fp, lhsT=coarse[cc].rearrange("p a b -> p (a b)"),
                             rhs=i_sb, start=True, stop=True)
            featp.append(fp)
        feat_sb = small.tile([G * G, C], f32)
        for cc in range(2):
            nc.scalar.copy(out=feat_sb[:, cc * 128:(cc + 1) * 128], in_=featp[cc])
        stats = small.tile([G * G, 6], f32)
        nc.vector.bn_stats(out=stats, in_=feat_sb)
        mv = small.tile([G * G, 2], f32)
        nc.vector.bn_aggr(out=mv, in_=stats)

        eps_sb = small.tile([G * G, 1], f32)
        nc.vector.memset(eps_sb, LN_EPS)
        zero_sb = small.tile([G * G, 1], f32)
        nc.vector.memset(zero_sb, 0.0)
        srt = small.tile([G * G, 1], f32)
        nc.scalar.activation(out=srt, in_=mv[:, 1:2],
                             func=mybir.ActivationFunctionType.Sqrt, bias=eps_sb)
        s_t = small.tile([G * G, 1], f32)
        nc.vector.reciprocal(out=s_t, in_=srt)
        ms = small.tile([G * G, 1], f32)
        nc.vector.tensor_mul(out=ms, in0=mv[:, 0:1], in1=s_t)
        t_t = small.tile([G * G, 1], f32)
        nc.vector.tensor_scalar_mul(out=t_t, in0=ms, scalar1=-1.0)
        mu2 = small.tile([G * G, 1], f32)
        nc.vector.tensor_mul(out=mu2, in0=mv[:, 0:1], in1=mv[:, 0:1])
        vm = small.tile([G * G, 1], f32)
        nc.vector.tensor_add(out=vm, in0=mv[:, 1:2], in1=mu2)
        imp = small.tile([G * G, 1], f32)
        nc.scalar.activation(out=imp, in_=vm,
                             func=mybir.ActivationFunctionType.Sqrt,
                             bias=zero_sb, scale=float(C))

        # ---- importance -> (1,16), min/max norm, threshold mask
        impTp = psum.tile([1, G * G], f32)
        nc.tensor.matmul(impTp, lhsT=imp, rhs=i_sb[:G * G, :G * G],
                         start=True, stop=True)
        impT = small.tile([1, G * G], f32)
        nc.scalar.copy(out=impT, in_=impTp)
        mn = small.tile([1, 1], f32)
        nc.vector.tensor_reduce(out=mn, in_=impT, axis=X, op=mybir.AluOpType.min)
        mxv = small.tile([1, 1], f32)
        nc.vector.tensor_reduce(out=mxv, in_=impT, axis=X, op=mx_op)
        rng_t = small.tile([1, 1], f32)
        nc.vector.tensor_sub(out=rng_t, in0=mxv, in1=mn)
        rngp = small.tile([1, 1], f32)
        nc.vector.tensor_scalar_add(out=rngp, in0=rng_t, scalar1=1e-8)
        rcp = small.tile([1, 1], f32)
        nc.vector.reciprocal(out=rcp, in_=rngp)
        imp01 = small.tile([1, G * G], f32)
        nc.vector.tensor_scalar(out=imp01, in0=impT, scalar1=mn, scalar2=rcp,
                                op0=mybir.AluOpType.subtract,
                                op1=mybir.AluOpType.mult)
        mask = small.tile([1, G * G], f32)
        nc.vector.tensor_scalar(out=mask, in0=imp01, scalar1=thr_sb, scalar2=None,
                                op0=mybir.AluOpType.is_ge)
        nc.sync.dma_start(out=keep_d.ap(), in_=mask)

        # ---- bilinear upsample mask: V = mask4 @ WxT, band = WyT^T @ V
        vp = psum.tile([G, W], f32)
        m4j = mask.rearrange("p (i j) -> p j i", j=G)
        for j in range(G):
            nc.tensor.matmul(vp, lhsT=m4j[:, j, :], rhs=wxT_sb[:, j, :],
                             start=(j == 0), stop=(j == G - 1))
        v_sb = small.tile([G, W], f32)
        nc.scalar.copy(out=v_sb, in_=vp)
        mbs = []
        for rc in range(2):
            mbp = psum.tile([RC, W], f32, name=f"mbp{rc}")
            nc.tensor.matmul(mbp, lhsT=wyT_sb[:, rc * RC:(rc + 1) * RC], rhs=v_sb,
                             start=True, stop=True)
            mb = small.tile([RC, W], f32, name=f"mb{rc}")
            nc.scalar.copy(out=mb, in_=mbp)
            mbs.append(mb)

        # ---- proj = s*(coarse^T @ wg) + t*A + Bw, then keep-mask it
        projp = psum.tile([G * G, OUT], f32)
        for cc in range(2):
            nc.tensor.matmul(projp, lhsT=coarse[cc].rearrange("p a b -> p (a b)"),
                             rhs=wg_sb[:, cc, :], start=(cc == 0), stop=(cc == 1))
        term = small.tile([G * G, OUT], f32)
        nc.vector.tensor_scalar(out=term, in0=a_sb, scalar1=t_t, scalar2=None,
                                op0=mybir.AluOpType.mult)
        proj1 = small.tile([G * G, OUT], f32)
        nc.vector.tensor_scalar(out=proj1, in0=projp, scalar1=s_t, scalar2=None,
                                op0=mybir.AluOpType.mult)
        proj2 = small.tile([G * G, OUT], f32)
        nc.vector.tensor_add(out=proj2, in0=proj1, in1=term)
        proj3 = small.tile([G * G, OUT], f32)
        nc.vector.tensor_add(out=proj3, in0=proj2, in1=b_sb)

        ones11 = small.tile([1, 1], f32)
        nc.vector.memset(ones11, 1.0)
        keepPp = psum.tile([G * G, 1], f32)
        nc.tensor.matmul(keepPp, lhsT=mask, rhs=ones11, start=True, stop=True)
        keepP = small.tile([G * G, 1], f32)
        nc.scalar.copy(out=keepP, in_=keepPp)
        kept = small.tile([G * G, OUT], f32)
        nc.vector.tensor_scalar(out=kept, in0=proj3, scalar1=keepP, scalar2=None,
                                op0=mybir.AluOpType.mult)
        nc.sync.dma_start(out=kept_d.ap(), in_=kept)

        # ---- pass 2: out = x * mask (row-partition layout)
        for rc in range(2):
            mb = mbs[rc]
            mb_b = bass.AP(tensor=mb.tensor, offset=mb.offset,
                           ap=[mb.ap[0], [0, csub], mb.ap[-1]])
            for cb in range(C // csub):
                x2 = p2.tile([RC, csub, W], f32, tag="x2", name="x2")
                src = x_ap[cb * csub:(cb + 1) * csub, rc * RC:(rc + 1) * RC, :]
                nc.sync.dma_start(out=x2, in_=src.rearrange("c r w -> r c w"))
                nc.vector.tensor_mul(out=x2, in0=x2, in1=mb_b)
                dst = out_ap[cb * csub:(cb + 1) * csub, rc * RC:(rc + 1) * RC, :]
                nc.sync.dma_start(out=dst.rearrange("c r w -> r c w"), in_=x2)

    nc.compile()
    return nc


def _get_nc():
    if "nc" not in _CACHE:
        _CACHE["nc"] = _build_nc()
    return _CACHE["nc"]


def _coords16():
    centers = (np.arange(G, dtype=np.float32) + np.float32(0.5)) / np.float32(G)
    out = np.empty((G * G, 4), np.float32)
    for n in range(G * G):
        out[n, 0] = centers[n % G]   # cx
        out[n, 1] = centers[n // G]  # cy
        out[n, 2] = np.float32(1.0) / np.float32(G)
        out[n, 3] = np.float32(1.0) / np.float32(G)
    return out


def _host_params(logit_thresholds, ln_gamma, ln_beta, w_proj, b_proj):
    lg = np.asarray(ln_gamma, np.float32)
    lb = np.asarray(ln_beta, np.float32)
    wp = np.asarray(w_proj, np.float32)
    bp = np.asarray(b_proj, np.float32)
    lt = np.asarray(logit_thresholds, np.float32)

    wg = np.ascontiguousarray(lg[:, None] * wp)                     # (C, OUT)
    a_vec = np.sum(wg.astype(np.float64), axis=0)
    bw = lb.astype(np.float64) @ wp.astype(np.float64) + bp.astype(np.float64)
    ab = np.ascontiguousarray(np.stack([a_vec, bw]).astype(np.float32))
    th = np.float32(1.0) / (np.float32(1.0) + np.exp(-lt[-1], dtype=np.float32))
    sc = np.array([[th]], np.float32)
    return wg, ab, sc


def kernel(x, logit_thresholds, ln_gamma, ln_beta, w_proj, b_proj,
           _trace=False, _trace_kwargs=None):
    from concourse.bass_utils import run_bass_kernel_spmd

    x = np.asarray(x, np.float32)
    assert x.shape == (B, C, H, W)
    wg, ab, sc = _host_params(logit_thresholds, ln_gamma, ln_beta, w_proj, b_proj)

    nc = _get_nc()
    in_maps = [{"x": np.ascontiguousarray(x[i]), "wg": wg, "ab": ab, "sc": sc}
               for i in range(N_CORES)]
    res = run_bass_kernel_spmd(nc, in_maps, core_ids=list(range(N_CORES)),
                               trace=_trace, **(_trace_kwargs or {}))
    outs = res.results

    out_sparse = np.stack([outs[i]["out"] for i in range(N_CORES)])
    kept_feats = np.stack([outs[i]["kept"] for i in range(N_CORES)])
    keep = np.stack([outs[i]["keep"].reshape(G * G) for i in range(N_CORES)])
    kept_coords = _coords16()[None, :, :] * keep[:, :, None]

    if _trace:
        kernel._last_results = res
    return out_sparse, kept_feats, kept_coords, keep
